# revision 6
# baseline (speedup 1.0000x reference)
"""EGNN EquivariantBlock kernel for 8x TRN2 NeuronCores (Bass/Tile).

Strategy:
  - Sort edges by dst (host). Shard edges across 8 cores at node boundaries
    -> each core owns a contiguous dst-node range; no collectives needed.
  - Per core: dst-node blocks of 128 nodes. Edges of a block are split into
    lo/hi groups by src (so int16 dma_gather indices fit), padded to a
    uniform number of 128-edge subtiles per block (SPMD: same NEFF all cores).
  - Gather h||x rows (512B) by src via dma_gather. Edge MLPs run in
    feature-transposed orientation (features on partitions, edges on free):
      pre = Wa^T h_srcT + zb(dst, expanded via one-hot S^T) + Wra^T [r; a]
    Segment-sum via one-hot matmul (lhsT = S edge-row) into PSUM agg.
  - ACT engine runs ONLY silu (no table switches): sigmoid = silu(z)/z via
    DVE reciprocal; sqrt via DVE Newton-rsqrt bit trick.
  - Node phase per block: u = silu(cat(h, h_neigh) @ Wn1 + b), outputs
    h_out = h + u @ Wn2 + b, x_out = x + (agg_gx - x * agg_g).
"""
import numpy as np
import ml_dtypes

import concourse.bass as bass
import concourse.bacc as bacc
import concourse.mybir as mybir
import concourse.tile as tile
from concourse.bass_utils import run_bass_kernel_spmd
from concourse.library_config import mlp as _mlp_lib
from concourse.masks import make_identity

BF16 = mybir.dt.bfloat16
F32 = mybir.dt.float32
I32 = mybir.dt.int32
I16 = mybir.dt.int16
bf16 = ml_dtypes.bfloat16

# problem constants (hardcoded per contract)
N, E, D, DE, DC = 50000, 800000, 128, 16, 3
NCORES = 8
P = 128
_LAST = None


# ---------------------------------------------------------------- host prep
def _ceil(a, b):
    return -(-a // b)


def _wrap_idx(flat_idx):
    """dma_gather layout A: [n] int16 -> [128, n//16] (16-wrap, x8 replicate)."""
    base = flat_idx.reshape(-1, 16).T  # [16, n/16]
    return np.tile(base, (8, 1)).astype(np.int16)


def prep(h, x, a, src, dst, weights, ncores=NCORES):
    """Build per-core device arrays + config. weights: dict of W_*/b_* f32."""
    n, d = h.shape
    e = src.shape[0]
    de = a.shape[1]
    npad = _ceil(n, 256) * 256
    nlo = npad // 2
    src = src.astype(np.int64)
    dst = dst.astype(np.int64)

    # hx gather table: rows of 512B: h bf16[128] | x f32[3] (as 6 bf16 slots) | pad
    rowlen = 256  # bf16 elements
    hx = np.zeros((npad, rowlen), dtype=bf16)
    hx[:n, :d] = h.astype(bf16)
    hx[:n, d:d + 2 * DC] = x.astype(np.float32).view(np.uint16).reshape(n, 2 * DC).view(bf16)
    hx_lo, hx_hi = np.ascontiguousarray(hx[:nlo]), np.ascontiguousarray(hx[nlo:])

    perm = np.argsort(dst, kind="stable")
    dsts = dst[perm]
    srcs = src[perm]
    # shard cuts at node boundaries, ~equal edges
    counts = np.bincount(dst, minlength=n)
    cum = np.concatenate([[0], np.cumsum(counts)])  # cum[i] = #edges with dst < i
    cuts = [0]
    for k in range(1, ncores):
        tgt = e * k // ncores
        c = int(np.searchsorted(cum, tgt))
        c = min(max(c, cuts[-1] + 1), n - (ncores - k))
        cuts.append(c)
    cuts.append(n)
    nblk = max(_ceil(cuts[k + 1] - cuts[k], P) for k in range(ncores))

    # per (core, block) edge groups
    blocks = []  # (core, b) -> (lo_edges_idx, hi_edges_idx) positions into perm arrays
    max_lo = max_hi = 0
    for k in range(ncores):
        lo_n, hi_n = cuts[k], cuts[k + 1]
        for b in range(nblk):
            nb0 = lo_n + b * P
            nb1 = min(nb0 + P, hi_n)
            if nb0 >= hi_n:
                e0 = e1 = cum[hi_n]
            else:
                e0, e1 = cum[nb0], cum[nb1]
            seg = np.arange(e0, e1)
            is_lo = srcs[e0:e1] < nlo
            lo_i = seg[is_lo]
            hi_i = seg[~is_lo]
            blocks.append((lo_i, hi_i))
            max_lo = max(max_lo, len(lo_i))
            max_hi = max(max_hi, len(hi_i))
    lo_t = max(1, _ceil(max_lo, P))
    hi_t = max(1, _ceil(max_hi, P))
    nsub = lo_t + hi_t

    # chunk schedule: groups of up to 4 subtiles
    chunks = []
    s = 0
    while s < nsub:
        s1 = min(s + 4, nsub)
        chunks.append((s, s1))
        s = s1
    cfg = dict(n=n, d=d, e=e, de=de, npad=npad, nlo=nlo, nblk=nblk,
               lo_t=lo_t, hi_t=hi_t, nsub=nsub, chunks=chunks, cuts=cuts,
               ncores=ncores)

    in_maps = []
    af = a.astype(np.float32)
    for k in range(ncores):
        lo_n, hi_n = cuts[k], cuts[k + 1]
        nn = nblk * P
        gil = np.zeros((nblk, 128, lo_t * 8), np.int16)
        gih = np.zeros((nblk, 128, hi_t * 8), np.int16)
        dlrow = np.full((nblk, 1, nsub * P), -1.0, np.float32)
        dlcol = np.full((nblk, 128, nsub), -1.0, np.float32)
        aTt = np.zeros((nblk, 16, nsub * P), bf16)
        for b in range(nblk):
            lo_i, hi_i = blocks[k * nblk + b]
            base = lo_n + b * P
            il = np.zeros(lo_t * P, np.int64)
            il[:len(lo_i)] = srcs[lo_i]
            ih = np.full(hi_t * P, nlo, np.int64)
            ih[:len(hi_i)] = srcs[hi_i]
            gil[b] = _wrap_idx(il.astype(np.int16))
            gih[b] = _wrap_idx((ih - nlo).astype(np.int16))
            dl = np.full(nsub * P, -1.0, np.float32)
            dl[:len(lo_i)] = dsts[lo_i] - base
            dl[lo_t * P:lo_t * P + len(hi_i)] = dsts[hi_i] - base
            dlrow[b, 0] = dl
            dlcol[b] = dl.reshape(nsub, P).T
            av = np.zeros((nsub * P, de), np.float32)
            av[:len(lo_i)] = af[perm[lo_i]]
            av[lo_t * P:lo_t * P + len(hi_i)] = af[perm[hi_i]]
            aTt[b] = av.T.astype(bf16)
        hTt = np.zeros((128, nn), bf16)
        nodef = np.zeros((nn, 132), np.float32)
        nreal = hi_n - lo_n
        hTt[:, :nreal] = h[lo_n:hi_n].T.astype(bf16)
        nodef[:nreal, :d] = h[lo_n:hi_n]
        nodef[:nreal, d:d + DC] = x[lo_n:hi_n]
        w = weights
        im = {
            "hx_lo": hx_lo, "hx_hi": hx_hi, "hTt": hTt, "nodef": nodef,
            "gil": gil, "gih": gih, "dlrow": dlrow, "dlcol": dlcol, "aTt": aTt,
            "Wa_e1": w["W_e1"][:d].astype(bf16), "Wb_e1": w["W_e1"][d:2 * d].astype(bf16),
            "Wr_e1": w["W_e1"][2 * d:2 * d + 1].astype(bf16),
            "Waa_e1": w["W_e1"][2 * d + 1:].astype(bf16),
            "Wa_c1": w["W_c1"][:d].astype(bf16), "Wb_c1": w["W_c1"][d:2 * d].astype(bf16),
            "Wr_c1": w["W_c1"][2 * d:2 * d + 1].astype(bf16),
            "Waa_c1": w["W_c1"][2 * d + 1:].astype(bf16),
            "W_e2": w["W_e2"].astype(bf16), "W_c2": w["W_c2"].astype(bf16),
            "W_att": w["W_att"].astype(bf16), "W_c3": w["W_c3"].astype(bf16),
            "Wn1a": w["W_n1"][:d].astype(bf16), "Wn1b": w["W_n1"][d:].astype(bf16),
            "W_n2": w["W_n2"].astype(bf16),
            "b_e1": w["b_e1"].reshape(d, 1).astype(np.float32),
            "b_c1": w["b_c1"].reshape(d, 1).astype(np.float32),
            "b_e2": w["b_e2"].reshape(d, 1).astype(np.float32),
            "b_c2": w["b_c2"].reshape(d, 1).astype(np.float32),
            "b_n1": w["b_n1"].reshape(d, 1).astype(np.float32),
            "b_att": np.float32(w["b_att"]).reshape(1, 1) + np.float32(1e-30),
            "b_n2": w["b_n2"].reshape(1, d).astype(np.float32),
        }
        in_maps.append(im)
    return cfg, in_maps


# ---------------------------------------------------------------- device build
def build(cfg):
    d = cfg["d"]
    de = cfg["de"]
    nblk, lo_t, hi_t, nsub = cfg["nblk"], cfg["lo_t"], cfg["hi_t"], cfg["nsub"]
    chunks = cfg["chunks"]
    nlo = cfg["nlo"]
    npad = cfg["npad"]
    nn = nblk * P
    AluOp = mybir.AluOpType
    SILU = mybir.ActivationFunctionType.Silu

    nc = bacc.Bacc("TRN2", target_bir_lowering=False, debug=False)
    dt_in = {
        "hx_lo": ([nlo, 256], BF16), "hx_hi": ([npad - nlo, 256], BF16),
        "hTt": ([128, nn], BF16), "nodef": ([nn, 132], F32),
        "gil": ([nblk, 128, lo_t * 8], I16), "gih": ([nblk, 128, hi_t * 8], I16),
        "dlrow": ([nblk, 1, nsub * P], F32), "dlcol": ([nblk, 128, nsub], F32),
        "aTt": ([nblk, 16, nsub * P], BF16),
        "Wa_e1": ([d, d], BF16), "Wb_e1": ([d, d], BF16), "Wr_e1": ([1, d], BF16),
        "Waa_e1": ([de, d], BF16),
        "Wa_c1": ([d, d], BF16), "Wb_c1": ([d, d], BF16), "Wr_c1": ([1, d], BF16),
        "Waa_c1": ([de, d], BF16),
        "W_e2": ([d, d], BF16), "W_c2": ([d, d], BF16),
        "W_att": ([d, 1], BF16), "W_c3": ([d, 1], BF16),
        "Wn1a": ([d, d], BF16), "Wn1b": ([d, d], BF16), "W_n2": ([d, d], BF16),
        "b_e1": ([d, 1], F32), "b_c1": ([d, 1], F32), "b_e2": ([d, 1], F32),
        "b_c2": ([d, 1], F32), "b_n1": ([d, 1], F32), "b_att": ([1, 1], F32),
        "b_n2": ([1, d], F32),
    }
    H = {k: nc.dram_tensor(k, shp, t, kind="ExternalInput") for k, (shp, t) in dt_in.items()}
    hout = nc.dram_tensor("hout", [nn, d], F32, kind="ExternalOutput")
    xout = nc.dram_tensor("xout", [nn, DC], F32, kind="ExternalOutput")

    with tile.TileContext(nc) as tc:
        with tc.tile_critical():
            nc.gpsimd.load_library(_mlp_lib)
        with tc.tile_pool(name="wp", bufs=1) as wp, \
             tc.tile_pool(name="sp", bufs=2) as sp, \
             tc.tile_pool(name="pp", bufs=2, space="PSUM") as pp:
            # ---- constants
            W = {}
            for k in ["Wa_e1", "Wb_e1", "Wr_e1", "Waa_e1", "Wa_c1", "Wb_c1",
                      "Wr_c1", "Waa_c1", "W_e2", "W_c2", "W_att", "W_c3",
                      "Wn1a", "Wn1b", "W_n2"]:
                t = wp.tile(dt_in[k][0], BF16, tag=k)
                nc.sync.dma_start(out=t[:], in_=H[k][:])
                W[k] = t
            B = {}
            for k in ["b_e1", "b_c1", "b_e2", "b_c2", "b_n1", "b_att"]:
                t = wp.tile(dt_in[k][0], F32, tag=k)
                nc.sync.dma_start(out=t[:], in_=H[k][:])
                B[k] = t
            b_att_col = wp.tile([128, 1], F32, tag="b_att_col")
            nc.gpsimd.partition_broadcast(b_att_col[:], B["b_att"][:])
            b_n2row = wp.tile([1, d], F32, tag="b_n2row")
            nc.sync.dma_start(out=b_n2row[:], in_=H["b_n2"][:])
            b_n2bc = wp.tile([128, d], F32, tag="b_n2bc")
            nc.gpsimd.partition_broadcast(b_n2bc[:], b_n2row[:])
            idn_b = wp.tile([128, 128], BF16, tag="idn_b")
            make_identity(nc, idn_b[:])
            idn_f = wp.tile([128, 128], F32, tag="idn_f")
            make_identity(nc, idn_f[:])
            idn33_f = wp.tile([33, 33], F32, tag="idn33_f")
            make_identity(nc, idn33_f[:])
            iota_c = wp.tile([128, 1], I32, tag="iota_c")
            nc.gpsimd.iota(iota_c[:], pattern=[[0, 1]], base=0, channel_multiplier=1)
            iota_cf = wp.tile([128, 1], F32, tag="iota_cf")
            nc.vector.tensor_copy(iota_cf[:], iota_c[:])
            iota_r = wp.tile([128, 128], I32, tag="iota_r")
            nc.gpsimd.iota(iota_r[:], pattern=[[1, 128]], base=0, channel_multiplier=0)
            iota_rf = wp.tile([128, 128], F32, tag="iota_rf")
            nc.vector.tensor_copy(iota_rf[:], iota_r[:])

            for b in range(nblk):
                # ---------------- stage A: node block prep
                hTblk = sp.tile([128, 128], BF16, tag="hTblk")
                nc.sync.dma_start(out=hTblk[:], in_=H["hTt"][:, b * P:(b + 1) * P])
                nodeblk = sp.tile([128, 132], F32, tag="nodeblk")
                nc.sync.dma_start(out=nodeblk[:], in_=H["nodef"][b * P:(b + 1) * P, :])
                xblk_b = sp.tile([128, DC], BF16, tag="xblk_b")
                nc.vector.tensor_copy(xblk_b[:], nodeblk[:, d:d + DC])
                zb_ps = pp.tile([128, 256], F32, tag="scr2", bufs=1)
                nc.tensor.matmul(out=zb_ps[:, 0:128], lhsT=hTblk[:], rhs=W["Wb_e1"][:],
                                 start=True, stop=True)
                nc.tensor.matmul(out=zb_ps[:, 128:256], lhsT=hTblk[:], rhs=W["Wb_c1"][:],
                                 start=True, stop=True)
                zbe = sp.tile([128, 128], BF16, tag="zbe")
                nc.vector.tensor_copy(zbe[:], zb_ps[:, 0:128])
                zbc = sp.tile([128, 128], BF16, tag="zbc")
                nc.vector.tensor_copy(zbc[:], zb_ps[:, 128:256])

                # ---------------- stage B: gather + geometry
                gi_l = sp.tile([128, lo_t * 8], I16, tag="gi_l")
                nc.sync.dma_start(out=gi_l[:], in_=H["gil"][b])
                gi_h = sp.tile([128, hi_t * 8], I16, tag="gi_h")
                nc.sync.dma_start(out=gi_h[:], in_=H["gih"][b])
                gat = sp.tile([128, nsub, 256], BF16, tag="gat")
                nc.gpsimd.dma_gather(
                    out_ap=gat[:, 0:lo_t, :], in_ap=H["hx_lo"][:], idxs_ap=gi_l[:],
                    num_idxs=lo_t * P, num_idxs_reg=lo_t * P, elem_size=256,
                    single_packet=(lo_t * P <= 1024))
                nc.gpsimd.dma_gather(
                    out_ap=gat[:, lo_t:nsub, :], in_ap=H["hx_hi"][:], idxs_ap=gi_h[:],
                    num_idxs=hi_t * P, num_idxs_reg=hi_t * P, elem_size=256,
                    single_packet=(hi_t * P <= 1024))
                dlr = sp.tile([1, nsub * P], F32, tag="dlr")
                nc.sync.dma_start(out=dlr[:], in_=H["dlrow"][b])
                dlc = sp.tile([128, nsub], F32, tag="dlc")
                nc.sync.dma_start(out=dlc[:], in_=H["dlcol"][b])
                aT = sp.tile([16, nsub * P], BF16, tag="aT")
                nc.sync.dma_start(out=aT[:], in_=H["aTt"][b])
                # S^T [nodes(P), edges] and S_sub [edges(P), nodes]
                dlb = sp.tile([128, nsub * P], F32, tag="dlb")
                nc.gpsimd.partition_broadcast(dlb[:], dlr[:])
                st_all = sp.tile([128, nsub * P], BF16, tag="st_all")
                nc.vector.tensor_scalar(out=st_all[:], in0=dlb[:],
                                        scalar1=iota_cf[:, 0:1], scalar2=None,
                                        op0=AluOp.is_equal)
                ssub = sp.tile([128, nsub, 128], BF16, tag="ssub")
                for t in range(nsub):
                    nc.vector.tensor_scalar(out=ssub[:, t, :], in0=iota_rf[:],
                                            scalar1=dlc[:, t:t + 1], scalar2=None,
                                            op0=AluOp.is_equal)
                # x_dst per subtile -> xdp psum [128, 3t]
                xdp = pp.tile([128, 3 * nsub], F32, tag="scr", bufs=1)
                for t in range(nsub):
                    nc.tensor.matmul(out=xdp[:, 3 * t:3 * t + 3],
                                     lhsT=st_all[:, t * P:(t + 1) * P], rhs=xblk_b[:],
                                     start=True, stop=True)
                rcols = sp.tile([128, nsub], F32, tag="rcols")
                xdsc = sp.tile([128, DC], F32, tag="xdsc")
                for t in range(nsub):
                    xs = gat[:, t, d:d + 2 * DC].bitcast(F32)
                    nc.vector.tensor_tensor(out=xdsc[:], in0=xs, in1=xdp[:, 3 * t:3 * t + 3],
                                            op=AluOp.subtract)
                    nc.vector.tensor_tensor(out=xdsc[:], in0=xdsc[:], in1=xdsc[:],
                                            op=AluOp.mult)
                    nc.vector.reduce_sum(rcols[:, t:t + 1], xdsc[:], axis=mybir.AxisListType.X)
                # radial = sqrt(rcols) via Newton rsqrt; invr = 1/(1+radial)
                rc = sp.tile([128, nsub], F32, tag="rc")
                nc.vector.tensor_scalar(out=rc[:], in0=rcols[:], scalar1=1e-20,
                                        scalar2=None, op0=AluOp.max)
                ybit = sp.tile([128, nsub], I32, tag="ybit")
                nc.vector.tensor_scalar(out=ybit[:], in0=rc[:].bitcast(I32), scalar1=1,
                                        scalar2=None, op0=AluOp.logical_shift_right)
                nc.vector.tensor_scalar(out=ybit[:], in0=ybit[:], scalar1=-1,
                                        scalar2=0x5f3759df, op0=AluOp.mult, op1=AluOp.add)
                ny = sp.tile([128, nsub], F32, tag="ny")
                nc.vector.tensor_copy(out=ny[:], in_=ybit[:].bitcast(F32))
                nxh = sp.tile([128, nsub], F32, tag="nxh")
                nc.vector.tensor_scalar(out=nxh[:], in0=rc[:], scalar1=0.5, scalar2=None,
                                        op0=AluOp.mult)
                nt = sp.tile([128, nsub], F32, tag="nt")
                for _ in range(3):
                    nc.vector.tensor_tensor(out=nt[:], in0=ny[:], in1=ny[:], op=AluOp.mult)
                    nc.vector.tensor_tensor(out=nt[:], in0=nt[:], in1=nxh[:], op=AluOp.mult)
                    nc.vector.tensor_scalar(out=nt[:], in0=nt[:], scalar1=-1.0, scalar2=1.5,
                                            op0=AluOp.mult, op1=AluOp.add)
                    nc.vector.tensor_tensor(out=ny[:], in0=ny[:], in1=nt[:], op=AluOp.mult)
                rad = sp.tile([128, nsub], F32, tag="rad")
                nc.vector.tensor_tensor(out=rad[:], in0=rcols[:], in1=ny[:], op=AluOp.mult)
                invr = sp.tile([128, nsub], F32, tag="invr")
                nc.vector.tensor_scalar(out=invr[:], in0=rad[:], scalar1=1.0, scalar2=None,
                                        op0=AluOp.add)
                nc.vector.reciprocal(invr[:], invr[:])
                radT = sp.tile([1, nsub * P], BF16, tag="radT")

                # agg psum for the whole block
                agg = pp.tile([128, 132], F32, tag="agg", bufs=1)
                first_seg = True

                # ---------------- stage C: MLP chunks
                for (s0, s1) in chunks:
                    ns = s1 - s0
                    ef = ns * P
                    hsT_ps = pp.tile([128, 4 * P], BF16, tag="trh", bufs=1)
                    for t in range(ns):
                        nc.tensor.transpose(out=hsT_ps[:, t * P:(t + 1) * P],
                                            in_=gat[:, s0 + t, 0:d], identity=idn_b[:])
                    hsrcT = sp.tile([128, 4 * P], BF16, tag="hsrcT")
                    nc.vector.tensor_copy(hsrcT[:, 0:ef], hsT_ps[:, 0:ef])
                    radT_ps = pp.tile([1, 4 * P], F32, tag="scr3", bufs=1)
                    for t in range(ns):
                        nc.tensor.transpose(out=radT_ps[0:1, t * P:(t + 1) * P],
                                            in_=rad[:, s0 + t:s0 + t + 1], identity=idn_f[:])
                    nc.vector.tensor_copy(radT[0:1, s0 * P:s1 * P], radT_ps[0:1, 0:ef])
                    for (nm, wa, zb, wr, waa, bb) in [
                            ("e1", "Wa_e1", zbe, "Wr_e1", "Waa_e1", "b_e1"),
                            ("c1", "Wa_c1", zbc, "Wr_c1", "Waa_c1", "b_c1")]:
                        pre = pp.tile([128, 4 * P], F32, tag="mlp", bufs=2)
                        nc.tensor.matmul(out=pre[:, 0:ef], lhsT=W[wa][:], rhs=hsrcT[:, 0:ef],
                                         start=True, stop=False)
                        nc.tensor.matmul(out=pre[:, 0:ef], lhsT=zb[:],
                                         rhs=st_all[:, s0 * P:s1 * P], start=False, stop=False)
                        nc.tensor.matmul(out=pre[:, 0:ef], lhsT=W[waa][:],
                                         rhs=aT[:, s0 * P:s1 * P], start=False, stop=False)
                        nc.tensor.matmul(out=pre[:, 0:ef], lhsT=W[wr][:],
                                         rhs=radT[0:1, s0 * P:s1 * P], start=False,
                                         stop=True)
                        mt = sp.tile([128, 4 * P], BF16, tag=f"m_{nm}")
                        nc.scalar.activation(mt[:, 0:ef], pre[:, 0:ef], SILU, bias=B[bb][:, 0:1])
                        if nm == "e1":
                            m1T = mt
                        else:
                            c1T = mt
                    z2 = pp.tile([128, 4 * P], F32, tag="mlp", bufs=2)
                    nc.tensor.matmul(out=z2[:, 0:ef], lhsT=W["W_e2"][:], rhs=m1T[:, 0:ef],
                                     start=True, stop=True)
                    msghT = sp.tile([128, 4 * P], BF16, tag="msghT")
                    nc.scalar.activation(msghT[:, 0:ef], z2[:, 0:ef], SILU, bias=B["b_e2"][:, 0:1])
                    z2c = pp.tile([128, 4 * P], F32, tag="mlp", bufs=2)
                    nc.tensor.matmul(out=z2c[:, 0:ef], lhsT=W["W_c2"][:], rhs=c1T[:, 0:ef],
                                     start=True, stop=True)
                    c2T = sp.tile([128, 4 * P], BF16, tag="c2T")
                    nc.scalar.activation(c2T[:, 0:ef], z2c[:, 0:ef], SILU, bias=B["b_c2"][:, 0:1])
                    # att raw + cw rows
                    ac_ps = pp.tile([33, 4 * P], F32, tag="scr3", bufs=1)
                    nc.tensor.matmul(out=ac_ps[0:1, 0:ef], lhsT=W["W_att"][:],
                                     rhs=msghT[:, 0:ef], start=True, stop=True)
                    nc.tensor.matmul(out=ac_ps[32:33, 0:ef], lhsT=W["W_c3"][:],
                                     rhs=c2T[:, 0:ef], start=True, stop=True)
                    pack2 = sp.tile([33, 4 * P], F32, tag="pack2")
                    nc.vector.tensor_copy(pack2[0:33, 0:ef], ac_ps[0:33, 0:ef])
                    p2T_ps = pp.tile([128, 4 * 33], F32, tag="scr2", bufs=1)
                    for t in range(ns):
                        nc.tensor.transpose(out=p2T_ps[:, 33 * t:33 * t + 33],
                                            in_=pack2[:, t * P:(t + 1) * P], identity=idn33_f[:])
                    # sigmoid(att+b) = silu(w)/w
                    wcol = sp.tile([128, 4], F32, tag="wcol")
                    nc.vector.tensor_scalar(out=wcol[:, 0:ns],
                                            in0=p2T_ps[:, 0:33 * ns:33],
                                            scalar1=b_att_col[:, 0:1],
                                            scalar2=None, op0=AluOp.add)
                    silw = sp.tile([128, 4], F32, tag="silw")
                    nc.scalar.activation(silw[:, 0:ns], wcol[:, 0:ns], SILU)
                    rw = sp.tile([128, 4], F32, tag="rw")
                    nc.vector.reciprocal(rw[:, 0:ns], wcol[:, 0:ns])
                    att = sp.tile([128, 4], F32, tag="att")
                    nc.vector.tensor_tensor(out=att[:, 0:ns], in0=silw[:, 0:ns],
                                            in1=rw[:, 0:ns], op=AluOp.mult)
                    gcol = sp.tile([128, 4], F32, tag="gcol")
                    nc.vector.tensor_tensor(out=gcol[:, 0:ns], in0=p2T_ps[:, 32:33 * ns:33],
                                            in1=invr[:, s0:s1], op=AluOp.mult)
                    # per subtile: transpose msg, gate, pack, segsum
                    mge_ps = pp.tile([128, 4 * P], BF16, tag="trm", bufs=1)
                    for t in range(ns):
                        nc.tensor.transpose(out=mge_ps[:, t * P:(t + 1) * P],
                                            in_=msghT[:, t * P:(t + 1) * P], identity=idn_b[:])
                    for t in range(ns):
                        gp = sp.tile([128, 132], BF16, tag="gp", bufs=4)
                        nc.vector.tensor_scalar(out=gp[:, 0:d],
                                                in0=mge_ps[:, t * P:(t + 1) * P],
                                                scalar1=att[:, t:t + 1], scalar2=None,
                                                op0=AluOp.mult)
                        xs = gat[:, s0 + t, d:d + 2 * DC].bitcast(F32)
                        nc.vector.tensor_scalar(out=gp[:, d:d + DC], in0=xs,
                                                scalar1=gcol[:, t:t + 1], scalar2=None,
                                                op0=AluOp.mult)
                        nc.vector.tensor_copy(gp[:, d + DC:d + DC + 1], gcol[:, t:t + 1])
                        nc.tensor.matmul(out=agg[:], lhsT=ssub[:, s0 + t, :], rhs=gp[:],
                                         start=first_seg, stop=(s0 + t == nsub - 1))
                        first_seg = False

                # ---------------- stage D: node update
                aggs = sp.tile([128, 132], F32, tag="aggs")
                nc.vector.tensor_copy(aggs[:], agg[:])
                hnb = sp.tile([128, 128], BF16, tag="hnb")
                nc.vector.tensor_copy(hnb[:], aggs[:, 0:d])
                hnT_ps = pp.tile([128, 128], BF16, tag="scr3", bufs=1)
                nc.tensor.transpose(out=hnT_ps[:], in_=hnb[:], identity=idn_b[:])
                hnT = sp.tile([128, 128], BF16, tag="hnT")
                nc.vector.tensor_copy(hnT[:], hnT_ps[:])
                upre = pp.tile([128, 128], F32, tag="scr", bufs=1)
                nc.tensor.matmul(out=upre[:], lhsT=W["Wn1a"][:], rhs=hTblk[:],
                                 start=True, stop=False)
                nc.tensor.matmul(out=upre[:], lhsT=W["Wn1b"][:], rhs=hnT[:],
                                 start=False, stop=True)
                uT = sp.tile([128, 128], BF16, tag="uT")
                nc.scalar.activation(uT[:], upre[:], SILU, bias=B["b_n1"][:, 0:1])
                hdel = pp.tile([128, 128], F32, tag="scr2", bufs=1)
                nc.tensor.matmul(out=hdel[:], lhsT=uT[:], rhs=W["W_n2"][:],
                                 start=True, stop=True)
                hout_s = sp.tile([128, 128], F32, tag="hout_s")
                nc.vector.tensor_tensor(out=hout_s[:], in0=hdel[:], in1=nodeblk[:, 0:d],
                                        op=AluOp.add)
                nc.vector.tensor_tensor(out=hout_s[:], in0=hout_s[:], in1=b_n2bc[:],
                                        op=AluOp.add)
                nc.sync.dma_start(out=hout[b * P:(b + 1) * P, :], in_=hout_s[:])
                xout_s = sp.tile([128, DC], F32, tag="xout_s")
                nc.vector.tensor_scalar(out=xout_s[:], in0=nodeblk[:, d:d + DC],
                                        scalar1=aggs[:, d + DC:d + DC + 1], scalar2=None,
                                        op0=AluOp.mult)
                nc.vector.tensor_tensor(out=xout_s[:], in0=aggs[:, d:d + DC], in1=xout_s[:],
                                        op=AluOp.subtract)
                nc.vector.tensor_tensor(out=xout_s[:], in0=xout_s[:], in1=nodeblk[:, d:d + DC],
                                        op=AluOp.add)
                nc.sync.dma_start(out=xout[b * P:(b + 1) * P, :], in_=xout_s[:])

    nc.compile()
    return nc


# ---------------------------------------------------------------- entry point
def kernel(h, x, a, src, dst,
           W_e1, b_e1, W_e2, b_e2, W_att, b_att,
           W_n1, b_n1, W_n2, b_n2,
           W_c1, b_c1, W_c2, b_c2, W_c3):
    h = np.asarray(h, np.float32)
    x = np.asarray(x, np.float32)
    a = np.asarray(a, np.float32)
    weights = dict(W_e1=np.asarray(W_e1, np.float32), b_e1=np.asarray(b_e1, np.float32),
                   W_e2=np.asarray(W_e2, np.float32), b_e2=np.asarray(b_e2, np.float32),
                   W_att=np.asarray(W_att, np.float32), b_att=np.asarray(b_att, np.float32),
                   W_n1=np.asarray(W_n1, np.float32), b_n1=np.asarray(b_n1, np.float32),
                   W_n2=np.asarray(W_n2, np.float32), b_n2=np.asarray(b_n2, np.float32),
                   W_c1=np.asarray(W_c1, np.float32), b_c1=np.asarray(b_c1, np.float32),
                   W_c2=np.asarray(W_c2, np.float32), b_c2=np.asarray(b_c2, np.float32),
                   W_c3=np.asarray(W_c3, np.float32))
    cfg, in_maps = prep(h, x, a, np.asarray(src), np.asarray(dst), weights)
    nc = build(cfg)
    global _LAST
    _LAST = (cfg, in_maps, nc)
    res = run_bass_kernel_spmd(nc, in_maps, core_ids=list(range(cfg["ncores"])))
    n, d = h.shape
    h_out = np.empty((n, d), np.float32)
    x_out = np.empty((n, DC), np.float32)
    cuts = cfg["cuts"]
    for k in range(cfg["ncores"]):
        lo_n, hi_n = cuts[k], cuts[k + 1]
        nreal = hi_n - lo_n
        h_out[lo_n:hi_n] = res.results[k]["hout"][:nreal]
        x_out[lo_n:hi_n] = res.results[k]["xout"][:nreal]
    return h_out, x_out


# revision 7
# speedup vs baseline: 990.0265x; 990.0265x over previous
"""EGNN EquivariantBlock kernel for 8x TRN2 NeuronCores (Bass/Tile).

Strategy:
  - Sort edges by dst (host). Shard edges across 8 cores at node boundaries
    -> each core owns a contiguous dst-node range; no collectives needed.
  - Per core: dst-node blocks of 128 nodes. Edges of a block are split into
    lo/hi groups by src (so int16 dma_gather indices fit), padded to a
    uniform number of 128-edge subtiles per block (SPMD: same NEFF all cores).
  - Gather h||x rows (512B) by src via dma_gather. Edge MLPs run in
    feature-transposed orientation (features on partitions, edges on free):
      pre = Wa^T h_srcT + zb(dst, expanded via one-hot S^T) + Wra^T [r; a]
    Segment-sum via one-hot matmul (lhsT = S edge-row) into PSUM agg.
  - ACT engine runs ONLY silu (no table switches): sigmoid = silu(z)/z via
    DVE reciprocal; sqrt via DVE Newton-rsqrt bit trick.
  - Node phase per block: u = silu(cat(h, h_neigh) @ Wn1 + b), outputs
    h_out = h + u @ Wn2 + b, x_out = x + (agg_gx - x * agg_g).
"""
import numpy as np
import ml_dtypes

import concourse.bass as bass
import concourse.bacc as bacc
import concourse.mybir as mybir
import concourse.tile as tile
from concourse.bass_utils import run_bass_kernel_spmd
from concourse.library_config import mlp as _mlp_lib
from concourse.masks import make_identity

BF16 = mybir.dt.bfloat16
F32 = mybir.dt.float32
I32 = mybir.dt.int32
I16 = mybir.dt.int16
bf16 = ml_dtypes.bfloat16

# problem constants (hardcoded per contract)
N, E, D, DE, DC = 50000, 800000, 128, 16, 3
NCORES = 8
P = 128
_LAST = None


# ---------------------------------------------------------------- host prep
def _ceil(a, b):
    return -(-a // b)


def _wrap_idx(flat_idx):
    """dma_gather layout A: [n] int16 -> [128, n//16] (16-wrap, x8 replicate)."""
    base = flat_idx.reshape(-1, 16).T  # [16, n/16]
    return np.tile(base, (8, 1)).astype(np.int16)


def prep(h, x, a, src, dst, weights, ncores=NCORES):
    """Build per-core device arrays + config. weights: dict of W_*/b_* f32."""
    n, d = h.shape
    e = src.shape[0]
    de = a.shape[1]
    npad = _ceil(n, 256) * 256
    nlo = npad // 2
    src = src.astype(np.int64)
    dst = dst.astype(np.int64)

    # hx gather table: rows of 512B: h bf16[128] | x f32[3] (as 6 bf16 slots) | pad
    rowlen = 256  # bf16 elements
    hx = np.zeros((npad, rowlen), dtype=bf16)
    hx[:n, :d] = h.astype(bf16)
    hx[:n, d:d + 2 * DC] = x.astype(np.float32).view(np.uint16).reshape(n, 2 * DC).view(bf16)
    hx_lo, hx_hi = np.ascontiguousarray(hx[:nlo]), np.ascontiguousarray(hx[nlo:])

    perm = np.argsort(dst, kind="stable")
    dsts = dst[perm]
    srcs = src[perm]
    # shard cuts at node boundaries, ~equal edges
    counts = np.bincount(dst, minlength=n)
    cum = np.concatenate([[0], np.cumsum(counts)])  # cum[i] = #edges with dst < i
    cuts = [0]
    for k in range(1, ncores):
        tgt = e * k // ncores
        c = int(np.searchsorted(cum, tgt))
        c = min(max(c, cuts[-1] + 1), n - (ncores - k))
        cuts.append(c)
    cuts.append(n)
    nblk = max(_ceil(cuts[k + 1] - cuts[k], P) for k in range(ncores))

    # per (core, block) edge groups
    blocks = []  # (core, b) -> (lo_edges_idx, hi_edges_idx) positions into perm arrays
    max_lo = max_hi = 0
    for k in range(ncores):
        lo_n, hi_n = cuts[k], cuts[k + 1]
        for b in range(nblk):
            nb0 = lo_n + b * P
            nb1 = min(nb0 + P, hi_n)
            if nb0 >= hi_n:
                e0 = e1 = cum[hi_n]
            else:
                e0, e1 = cum[nb0], cum[nb1]
            seg = np.arange(e0, e1)
            is_lo = srcs[e0:e1] < nlo
            lo_i = seg[is_lo]
            hi_i = seg[~is_lo]
            blocks.append((lo_i, hi_i))
            max_lo = max(max_lo, len(lo_i))
            max_hi = max(max_hi, len(hi_i))
    lo_t = max(1, _ceil(max_lo, P))
    hi_t = max(1, _ceil(max_hi, P))
    nsub = lo_t + hi_t

    # chunk schedule: groups of up to 4 subtiles
    chunks = []
    s = 0
    while s < nsub:
        s1 = min(s + 4, nsub)
        chunks.append((s, s1))
        s = s1
    cfg = dict(n=n, d=d, e=e, de=de, npad=npad, nlo=nlo, nblk=nblk,
               lo_t=lo_t, hi_t=hi_t, nsub=nsub, chunks=chunks, cuts=cuts,
               ncores=ncores)

    in_maps = []
    af = a.astype(np.float32)
    for k in range(ncores):
        lo_n, hi_n = cuts[k], cuts[k + 1]
        nn = nblk * P
        gil = np.zeros((nblk, 128, lo_t * 8), np.int16)
        gih = np.zeros((nblk, 128, hi_t * 8), np.int16)
        dlrow = np.full((nblk, 1, nsub * P), -1.0, np.float32)
        dlcol = np.full((nblk, 128, nsub), -1.0, np.float32)
        aTt = np.zeros((nblk, 16, nsub * P), bf16)
        for b in range(nblk):
            lo_i, hi_i = blocks[k * nblk + b]
            base = lo_n + b * P
            il = np.zeros(lo_t * P, np.int64)
            il[:len(lo_i)] = srcs[lo_i]
            ih = np.full(hi_t * P, nlo, np.int64)
            ih[:len(hi_i)] = srcs[hi_i]
            gil[b] = _wrap_idx(il.astype(np.int16))
            gih[b] = _wrap_idx((ih - nlo).astype(np.int16))
            dl = np.full(nsub * P, -1.0, np.float32)
            dl[:len(lo_i)] = dsts[lo_i] - base
            dl[lo_t * P:lo_t * P + len(hi_i)] = dsts[hi_i] - base
            dlrow[b, 0] = dl
            dlcol[b] = dl.reshape(nsub, P).T
            av = np.zeros((nsub * P, de), np.float32)
            av[:len(lo_i)] = af[perm[lo_i]]
            av[lo_t * P:lo_t * P + len(hi_i)] = af[perm[hi_i]]
            aTt[b] = av.T.astype(bf16)
        hTt = np.zeros((128, nn), bf16)
        nodef = np.zeros((nn, 132), np.float32)
        nreal = hi_n - lo_n
        hTt[:, :nreal] = h[lo_n:hi_n].T.astype(bf16)
        nodef[:nreal, :d] = h[lo_n:hi_n]
        nodef[:nreal, d:d + DC] = x[lo_n:hi_n]
        w = weights
        im = {
            "hx_lo": hx_lo, "hx_hi": hx_hi, "hTt": hTt, "nodef": nodef,
            "gil": gil, "gih": gih, "dlrow": dlrow, "dlcol": dlcol, "aTt": aTt,
            "Wa_e1": w["W_e1"][:d].astype(bf16), "Wb_e1": w["W_e1"][d:2 * d].astype(bf16),
            "Wr_e1": w["W_e1"][2 * d:2 * d + 1].astype(bf16),
            "Waa_e1": w["W_e1"][2 * d + 1:].astype(bf16),
            "Wa_c1": w["W_c1"][:d].astype(bf16), "Wb_c1": w["W_c1"][d:2 * d].astype(bf16),
            "Wr_c1": w["W_c1"][2 * d:2 * d + 1].astype(bf16),
            "Waa_c1": w["W_c1"][2 * d + 1:].astype(bf16),
            "W_e2": w["W_e2"].astype(bf16), "W_c2": w["W_c2"].astype(bf16),
            "W_att": w["W_att"].astype(bf16), "W_c3": w["W_c3"].astype(bf16),
            "Wn1a": w["W_n1"][:d].astype(bf16), "Wn1b": w["W_n1"][d:].astype(bf16),
            "W_n2": w["W_n2"].astype(bf16),
            "b_e1": w["b_e1"].reshape(d, 1).astype(np.float32),
            "b_c1": w["b_c1"].reshape(d, 1).astype(np.float32),
            "b_e2": w["b_e2"].reshape(d, 1).astype(np.float32),
            "b_c2": w["b_c2"].reshape(d, 1).astype(np.float32),
            "b_n1": w["b_n1"].reshape(d, 1).astype(np.float32),
            "b_att": np.float32(w["b_att"]).reshape(1, 1) + np.float32(1e-30),
            "b_n2": w["b_n2"].reshape(1, d).astype(np.float32),
        }
        in_maps.append(im)
    return cfg, in_maps


# ---------------------------------------------------------------- device build
def build(cfg, repeat=1):
    d = cfg["d"]
    de = cfg["de"]
    nblk, lo_t, hi_t, nsub = cfg["nblk"], cfg["lo_t"], cfg["hi_t"], cfg["nsub"]
    chunks = cfg["chunks"]
    nlo = cfg["nlo"]
    npad = cfg["npad"]
    nn = nblk * P
    AluOp = mybir.AluOpType
    SILU = mybir.ActivationFunctionType.Silu

    nc = bacc.Bacc("TRN2", target_bir_lowering=False, debug=False)
    dt_in = {
        "hx_lo": ([nlo, 256], BF16), "hx_hi": ([npad - nlo, 256], BF16),
        "hTt": ([128, nn], BF16), "nodef": ([nn, 132], F32),
        "gil": ([nblk, 128, lo_t * 8], I16), "gih": ([nblk, 128, hi_t * 8], I16),
        "dlrow": ([nblk, 1, nsub * P], F32), "dlcol": ([nblk, 128, nsub], F32),
        "aTt": ([nblk, 16, nsub * P], BF16),
        "Wa_e1": ([d, d], BF16), "Wb_e1": ([d, d], BF16), "Wr_e1": ([1, d], BF16),
        "Waa_e1": ([de, d], BF16),
        "Wa_c1": ([d, d], BF16), "Wb_c1": ([d, d], BF16), "Wr_c1": ([1, d], BF16),
        "Waa_c1": ([de, d], BF16),
        "W_e2": ([d, d], BF16), "W_c2": ([d, d], BF16),
        "W_att": ([d, 1], BF16), "W_c3": ([d, 1], BF16),
        "Wn1a": ([d, d], BF16), "Wn1b": ([d, d], BF16), "W_n2": ([d, d], BF16),
        "b_e1": ([d, 1], F32), "b_c1": ([d, 1], F32), "b_e2": ([d, 1], F32),
        "b_c2": ([d, 1], F32), "b_n1": ([d, 1], F32), "b_att": ([1, 1], F32),
        "b_n2": ([1, d], F32),
    }
    H = {k: nc.dram_tensor(k, shp, t, kind="ExternalInput") for k, (shp, t) in dt_in.items()}
    hout = nc.dram_tensor("hout", [nn, d], F32, kind="ExternalOutput")
    xout = nc.dram_tensor("xout", [nn, DC], F32, kind="ExternalOutput")

    with tile.TileContext(nc) as tc:
        with tc.tile_critical():
            nc.gpsimd.load_library(_mlp_lib)
        with tc.tile_pool(name="wp", bufs=1) as wp, \
             tc.tile_pool(name="sp", bufs=2) as sp, \
             tc.tile_pool(name="pp", bufs=2, space="PSUM") as pp:
            # ---- constants
            W = {}
            for k in ["Wa_e1", "Wb_e1", "Wr_e1", "Waa_e1", "Wa_c1", "Wb_c1",
                      "Wr_c1", "Waa_c1", "W_e2", "W_c2", "W_att", "W_c3",
                      "Wn1a", "Wn1b", "W_n2"]:
                t = wp.tile(dt_in[k][0], BF16, tag=k)
                nc.sync.dma_start(out=t[:], in_=H[k][:])
                W[k] = t
            B = {}
            for k in ["b_e1", "b_c1", "b_e2", "b_c2", "b_n1", "b_att"]:
                t = wp.tile(dt_in[k][0], F32, tag=k)
                nc.sync.dma_start(out=t[:], in_=H[k][:])
                B[k] = t
            b_att_col = wp.tile([128, 1], F32, tag="b_att_col")
            nc.gpsimd.partition_broadcast(b_att_col[:], B["b_att"][:])
            b_n2row = wp.tile([1, d], F32, tag="b_n2row")
            nc.sync.dma_start(out=b_n2row[:], in_=H["b_n2"][:])
            b_n2bc = wp.tile([128, d], F32, tag="b_n2bc")
            nc.gpsimd.partition_broadcast(b_n2bc[:], b_n2row[:])
            idn_b = wp.tile([128, 128], BF16, tag="idn_b")
            make_identity(nc, idn_b[:])
            idn_f = wp.tile([128, 128], F32, tag="idn_f")
            make_identity(nc, idn_f[:])
            idn33_f = wp.tile([33, 33], F32, tag="idn33_f")
            make_identity(nc, idn33_f[:])
            iota_c = wp.tile([128, 1], I32, tag="iota_c")
            nc.gpsimd.iota(iota_c[:], pattern=[[0, 1]], base=0, channel_multiplier=1)
            iota_cf = wp.tile([128, 1], F32, tag="iota_cf")
            nc.vector.tensor_copy(iota_cf[:], iota_c[:])
            iota_r = wp.tile([128, 128], I32, tag="iota_r")
            nc.gpsimd.iota(iota_r[:], pattern=[[1, 128]], base=0, channel_multiplier=0)
            iota_rf = wp.tile([128, 128], F32, tag="iota_rf")
            nc.vector.tensor_copy(iota_rf[:], iota_r[:])

            import contextlib
            loop_ctx = tc.For_i(0, repeat, 1) if repeat > 1 else contextlib.nullcontext()
            with loop_ctx:
              for b in range(nblk):
                # ---------------- stage A: node block prep
                hTblk = sp.tile([128, 128], BF16, tag="hTblk")
                nc.sync.dma_start(out=hTblk[:], in_=H["hTt"][:, b * P:(b + 1) * P])
                nodeblk = sp.tile([128, 132], F32, tag="nodeblk")
                nc.sync.dma_start(out=nodeblk[:], in_=H["nodef"][b * P:(b + 1) * P, :])
                xblk_b = sp.tile([128, DC], BF16, tag="xblk_b")
                nc.vector.tensor_copy(xblk_b[:], nodeblk[:, d:d + DC])
                zb_ps = pp.tile([128, 256], F32, tag="scr2", bufs=1)
                nc.tensor.matmul(out=zb_ps[:, 0:128], lhsT=hTblk[:], rhs=W["Wb_e1"][:],
                                 start=True, stop=True)
                nc.tensor.matmul(out=zb_ps[:, 128:256], lhsT=hTblk[:], rhs=W["Wb_c1"][:],
                                 start=True, stop=True)
                zbe = sp.tile([128, 128], BF16, tag="zbe")
                nc.vector.tensor_copy(zbe[:], zb_ps[:, 0:128])
                zbc = sp.tile([128, 128], BF16, tag="zbc")
                nc.vector.tensor_copy(zbc[:], zb_ps[:, 128:256])

                # ---------------- stage B: gather + geometry
                gi_l = sp.tile([128, lo_t * 8], I16, tag="gi_l")
                nc.sync.dma_start(out=gi_l[:], in_=H["gil"][b])
                gi_h = sp.tile([128, hi_t * 8], I16, tag="gi_h")
                nc.sync.dma_start(out=gi_h[:], in_=H["gih"][b])
                gat = sp.tile([128, nsub, 256], BF16, tag="gat")
                nc.gpsimd.dma_gather(
                    out_ap=gat[:, 0:lo_t, :], in_ap=H["hx_lo"][:], idxs_ap=gi_l[:],
                    num_idxs=lo_t * P, num_idxs_reg=lo_t * P, elem_size=256,
                    single_packet=(lo_t * P <= 1024))
                nc.gpsimd.dma_gather(
                    out_ap=gat[:, lo_t:nsub, :], in_ap=H["hx_hi"][:], idxs_ap=gi_h[:],
                    num_idxs=hi_t * P, num_idxs_reg=hi_t * P, elem_size=256,
                    single_packet=(hi_t * P <= 1024))
                dlr = sp.tile([1, nsub * P], F32, tag="dlr")
                nc.sync.dma_start(out=dlr[:], in_=H["dlrow"][b])
                dlc = sp.tile([128, nsub], F32, tag="dlc")
                nc.sync.dma_start(out=dlc[:], in_=H["dlcol"][b])
                aT = sp.tile([16, nsub * P], BF16, tag="aT")
                nc.sync.dma_start(out=aT[:], in_=H["aTt"][b])
                # S^T [nodes(P), edges] and S_sub [edges(P), nodes]
                dlb = sp.tile([128, nsub * P], F32, tag="dlb")
                nc.gpsimd.partition_broadcast(dlb[:], dlr[:])
                st_all = sp.tile([128, nsub * P], BF16, tag="st_all")
                nc.vector.tensor_scalar(out=st_all[:], in0=dlb[:],
                                        scalar1=iota_cf[:, 0:1], scalar2=None,
                                        op0=AluOp.is_equal)
                ssub = sp.tile([128, nsub, 128], BF16, tag="ssub")
                for t in range(nsub):
                    nc.vector.tensor_scalar(out=ssub[:, t, :], in0=iota_rf[:],
                                            scalar1=dlc[:, t:t + 1], scalar2=None,
                                            op0=AluOp.is_equal)
                # x_dst per subtile -> xdp psum [128, 3t]
                xdp = pp.tile([128, 3 * nsub], F32, tag="scr", bufs=1)
                for t in range(nsub):
                    nc.tensor.matmul(out=xdp[:, 3 * t:3 * t + 3],
                                     lhsT=st_all[:, t * P:(t + 1) * P], rhs=xblk_b[:],
                                     start=True, stop=True)
                rcols = sp.tile([128, nsub], F32, tag="rcols")
                xdsc = sp.tile([128, DC], F32, tag="xdsc")
                for t in range(nsub):
                    xs = gat[:, t, d:d + 2 * DC].bitcast(F32)
                    nc.vector.tensor_tensor(out=xdsc[:], in0=xs, in1=xdp[:, 3 * t:3 * t + 3],
                                            op=AluOp.subtract)
                    nc.vector.tensor_tensor(out=xdsc[:], in0=xdsc[:], in1=xdsc[:],
                                            op=AluOp.mult)
                    nc.vector.reduce_sum(rcols[:, t:t + 1], xdsc[:], axis=mybir.AxisListType.X)
                # radial = sqrt(rcols) via Newton rsqrt; invr = 1/(1+radial)
                rc = sp.tile([128, nsub], F32, tag="rc")
                nc.vector.tensor_scalar(out=rc[:], in0=rcols[:], scalar1=1e-20,
                                        scalar2=None, op0=AluOp.max)
                ybit = sp.tile([128, nsub], I32, tag="ybit")
                nc.vector.tensor_scalar(out=ybit[:], in0=rc[:].bitcast(I32), scalar1=1,
                                        scalar2=None, op0=AluOp.logical_shift_right)
                nc.vector.tensor_scalar(out=ybit[:], in0=ybit[:], scalar1=-1,
                                        scalar2=0x5f3759df, op0=AluOp.mult, op1=AluOp.add)
                ny = sp.tile([128, nsub], F32, tag="ny")
                nc.vector.tensor_copy(out=ny[:], in_=ybit[:].bitcast(F32))
                nxh = sp.tile([128, nsub], F32, tag="nxh")
                nc.vector.tensor_scalar(out=nxh[:], in0=rc[:], scalar1=0.5, scalar2=None,
                                        op0=AluOp.mult)
                nt = sp.tile([128, nsub], F32, tag="nt")
                for _ in range(3):
                    nc.vector.tensor_tensor(out=nt[:], in0=ny[:], in1=ny[:], op=AluOp.mult)
                    nc.vector.tensor_tensor(out=nt[:], in0=nt[:], in1=nxh[:], op=AluOp.mult)
                    nc.vector.tensor_scalar(out=nt[:], in0=nt[:], scalar1=-1.0, scalar2=1.5,
                                            op0=AluOp.mult, op1=AluOp.add)
                    nc.vector.tensor_tensor(out=ny[:], in0=ny[:], in1=nt[:], op=AluOp.mult)
                rad = sp.tile([128, nsub], F32, tag="rad")
                nc.vector.tensor_tensor(out=rad[:], in0=rcols[:], in1=ny[:], op=AluOp.mult)
                invr = sp.tile([128, nsub], F32, tag="invr")
                nc.vector.tensor_scalar(out=invr[:], in0=rad[:], scalar1=1.0, scalar2=None,
                                        op0=AluOp.add)
                nc.vector.reciprocal(invr[:], invr[:])
                radT = sp.tile([1, nsub * P], BF16, tag="radT")

                # agg psum for the whole block
                agg = pp.tile([128, 132], F32, tag="agg", bufs=1)
                first_seg = True

                # ---------------- stage C: MLP chunks
                for (s0, s1) in chunks:
                    ns = s1 - s0
                    ef = ns * P
                    hsT_ps = pp.tile([128, 4 * P], BF16, tag="trh", bufs=1)
                    for t in range(ns):
                        nc.tensor.transpose(out=hsT_ps[:, t * P:(t + 1) * P],
                                            in_=gat[:, s0 + t, 0:d], identity=idn_b[:])
                    hsrcT = sp.tile([128, 4 * P], BF16, tag="hsrcT")
                    nc.vector.tensor_copy(hsrcT[:, 0:ef], hsT_ps[:, 0:ef])
                    radT_ps = pp.tile([1, 4 * P], F32, tag="scr3", bufs=1)
                    for t in range(ns):
                        nc.tensor.transpose(out=radT_ps[0:1, t * P:(t + 1) * P],
                                            in_=rad[:, s0 + t:s0 + t + 1], identity=idn_f[:])
                    nc.vector.tensor_copy(radT[0:1, s0 * P:s1 * P], radT_ps[0:1, 0:ef])
                    for (nm, wa, zb, wr, waa, bb) in [
                            ("e1", "Wa_e1", zbe, "Wr_e1", "Waa_e1", "b_e1"),
                            ("c1", "Wa_c1", zbc, "Wr_c1", "Waa_c1", "b_c1")]:
                        pre = pp.tile([128, 4 * P], F32, tag="mlp", bufs=2)
                        nc.tensor.matmul(out=pre[:, 0:ef], lhsT=W[wa][:], rhs=hsrcT[:, 0:ef],
                                         start=True, stop=False)
                        nc.tensor.matmul(out=pre[:, 0:ef], lhsT=zb[:],
                                         rhs=st_all[:, s0 * P:s1 * P], start=False, stop=False)
                        nc.tensor.matmul(out=pre[:, 0:ef], lhsT=W[waa][:],
                                         rhs=aT[:, s0 * P:s1 * P], start=False, stop=False)
                        nc.tensor.matmul(out=pre[:, 0:ef], lhsT=W[wr][:],
                                         rhs=radT[0:1, s0 * P:s1 * P], start=False,
                                         stop=True)
                        mt = sp.tile([128, 4 * P], BF16, tag=f"m_{nm}")
                        nc.scalar.activation(mt[:, 0:ef], pre[:, 0:ef], SILU, bias=B[bb][:, 0:1])
                        if nm == "e1":
                            m1T = mt
                        else:
                            c1T = mt
                    z2 = pp.tile([128, 4 * P], F32, tag="mlp", bufs=2)
                    nc.tensor.matmul(out=z2[:, 0:ef], lhsT=W["W_e2"][:], rhs=m1T[:, 0:ef],
                                     start=True, stop=True)
                    msghT = sp.tile([128, 4 * P], BF16, tag="msghT")
                    nc.scalar.activation(msghT[:, 0:ef], z2[:, 0:ef], SILU, bias=B["b_e2"][:, 0:1])
                    z2c = pp.tile([128, 4 * P], F32, tag="mlp", bufs=2)
                    nc.tensor.matmul(out=z2c[:, 0:ef], lhsT=W["W_c2"][:], rhs=c1T[:, 0:ef],
                                     start=True, stop=True)
                    c2T = sp.tile([128, 4 * P], BF16, tag="c2T")
                    nc.scalar.activation(c2T[:, 0:ef], z2c[:, 0:ef], SILU, bias=B["b_c2"][:, 0:1])
                    # att raw + cw rows
                    ac_ps = pp.tile([33, 4 * P], F32, tag="scr3", bufs=1)
                    nc.tensor.matmul(out=ac_ps[0:1, 0:ef], lhsT=W["W_att"][:],
                                     rhs=msghT[:, 0:ef], start=True, stop=True)
                    nc.tensor.matmul(out=ac_ps[32:33, 0:ef], lhsT=W["W_c3"][:],
                                     rhs=c2T[:, 0:ef], start=True, stop=True)
                    pack2 = sp.tile([33, 4 * P], F32, tag="pack2")
                    nc.vector.tensor_copy(pack2[0:33, 0:ef], ac_ps[0:33, 0:ef])
                    p2T_ps = pp.tile([128, 4 * 33], F32, tag="scr2", bufs=1)
                    for t in range(ns):
                        nc.tensor.transpose(out=p2T_ps[:, 33 * t:33 * t + 33],
                                            in_=pack2[:, t * P:(t + 1) * P], identity=idn33_f[:])
                    # sigmoid(att+b) = silu(w)/w
                    wcol = sp.tile([128, 4], F32, tag="wcol")
                    nc.vector.tensor_scalar(out=wcol[:, 0:ns],
                                            in0=p2T_ps[:, 0:33 * ns:33],
                                            scalar1=b_att_col[:, 0:1],
                                            scalar2=None, op0=AluOp.add)
                    silw = sp.tile([128, 4], F32, tag="silw")
                    nc.scalar.activation(silw[:, 0:ns], wcol[:, 0:ns], SILU)
                    rw = sp.tile([128, 4], F32, tag="rw")
                    nc.vector.reciprocal(rw[:, 0:ns], wcol[:, 0:ns])
                    att = sp.tile([128, 4], F32, tag="att")
                    nc.vector.tensor_tensor(out=att[:, 0:ns], in0=silw[:, 0:ns],
                                            in1=rw[:, 0:ns], op=AluOp.mult)
                    gcol = sp.tile([128, 4], F32, tag="gcol")
                    nc.vector.tensor_tensor(out=gcol[:, 0:ns], in0=p2T_ps[:, 32:33 * ns:33],
                                            in1=invr[:, s0:s1], op=AluOp.mult)
                    # per subtile: transpose msg, gate, pack, segsum
                    mge_ps = pp.tile([128, 4 * P], BF16, tag="trm", bufs=1)
                    for t in range(ns):
                        nc.tensor.transpose(out=mge_ps[:, t * P:(t + 1) * P],
                                            in_=msghT[:, t * P:(t + 1) * P], identity=idn_b[:])
                    for t in range(ns):
                        gp = sp.tile([128, 132], BF16, tag="gp", bufs=4)
                        nc.vector.tensor_scalar(out=gp[:, 0:d],
                                                in0=mge_ps[:, t * P:(t + 1) * P],
                                                scalar1=att[:, t:t + 1], scalar2=None,
                                                op0=AluOp.mult)
                        xs = gat[:, s0 + t, d:d + 2 * DC].bitcast(F32)
                        nc.vector.tensor_scalar(out=gp[:, d:d + DC], in0=xs,
                                                scalar1=gcol[:, t:t + 1], scalar2=None,
                                                op0=AluOp.mult)
                        nc.vector.tensor_copy(gp[:, d + DC:d + DC + 1], gcol[:, t:t + 1])
                        nc.tensor.matmul(out=agg[:], lhsT=ssub[:, s0 + t, :], rhs=gp[:],
                                         start=first_seg, stop=(s0 + t == nsub - 1))
                        first_seg = False

                # ---------------- stage D: node update
                aggs = sp.tile([128, 132], F32, tag="aggs")
                nc.vector.tensor_copy(aggs[:], agg[:])
                hnb = sp.tile([128, 128], BF16, tag="hnb")
                nc.vector.tensor_copy(hnb[:], aggs[:, 0:d])
                hnT_ps = pp.tile([128, 128], BF16, tag="scr3", bufs=1)
                nc.tensor.transpose(out=hnT_ps[:], in_=hnb[:], identity=idn_b[:])
                hnT = sp.tile([128, 128], BF16, tag="hnT")
                nc.vector.tensor_copy(hnT[:], hnT_ps[:])
                upre = pp.tile([128, 128], F32, tag="scr", bufs=1)
                nc.tensor.matmul(out=upre[:], lhsT=W["Wn1a"][:], rhs=hTblk[:],
                                 start=True, stop=False)
                nc.tensor.matmul(out=upre[:], lhsT=W["Wn1b"][:], rhs=hnT[:],
                                 start=False, stop=True)
                uT = sp.tile([128, 128], BF16, tag="uT")
                nc.scalar.activation(uT[:], upre[:], SILU, bias=B["b_n1"][:, 0:1])
                hdel = pp.tile([128, 128], F32, tag="scr2", bufs=1)
                nc.tensor.matmul(out=hdel[:], lhsT=uT[:], rhs=W["W_n2"][:],
                                 start=True, stop=True)
                hout_s = sp.tile([128, 128], F32, tag="hout_s")
                nc.vector.tensor_tensor(out=hout_s[:], in0=hdel[:], in1=nodeblk[:, 0:d],
                                        op=AluOp.add)
                nc.vector.tensor_tensor(out=hout_s[:], in0=hout_s[:], in1=b_n2bc[:],
                                        op=AluOp.add)
                nc.sync.dma_start(out=hout[b * P:(b + 1) * P, :], in_=hout_s[:])
                xout_s = sp.tile([128, DC], F32, tag="xout_s")
                nc.vector.tensor_scalar(out=xout_s[:], in0=nodeblk[:, d:d + DC],
                                        scalar1=aggs[:, d + DC:d + DC + 1], scalar2=None,
                                        op0=AluOp.mult)
                nc.vector.tensor_tensor(out=xout_s[:], in0=aggs[:, d:d + DC], in1=xout_s[:],
                                        op=AluOp.subtract)
                nc.vector.tensor_tensor(out=xout_s[:], in0=xout_s[:], in1=nodeblk[:, d:d + DC],
                                        op=AluOp.add)
                nc.sync.dma_start(out=xout[b * P:(b + 1) * P, :], in_=xout_s[:])

    nc.compile()
    return nc


# ---------------------------------------------------------------- entry point
def kernel(h, x, a, src, dst,
           W_e1, b_e1, W_e2, b_e2, W_att, b_att,
           W_n1, b_n1, W_n2, b_n2,
           W_c1, b_c1, W_c2, b_c2, W_c3):
    h = np.asarray(h, np.float32)
    x = np.asarray(x, np.float32)
    a = np.asarray(a, np.float32)
    weights = dict(W_e1=np.asarray(W_e1, np.float32), b_e1=np.asarray(b_e1, np.float32),
                   W_e2=np.asarray(W_e2, np.float32), b_e2=np.asarray(b_e2, np.float32),
                   W_att=np.asarray(W_att, np.float32), b_att=np.asarray(b_att, np.float32),
                   W_n1=np.asarray(W_n1, np.float32), b_n1=np.asarray(b_n1, np.float32),
                   W_n2=np.asarray(W_n2, np.float32), b_n2=np.asarray(b_n2, np.float32),
                   W_c1=np.asarray(W_c1, np.float32), b_c1=np.asarray(b_c1, np.float32),
                   W_c2=np.asarray(W_c2, np.float32), b_c2=np.asarray(b_c2, np.float32),
                   W_c3=np.asarray(W_c3, np.float32))
    cfg, in_maps = prep(h, x, a, np.asarray(src), np.asarray(dst), weights)
    nc = build(cfg)
    global _LAST
    _LAST = (cfg, in_maps, nc)
    res = run_bass_kernel_spmd(nc, in_maps, core_ids=list(range(cfg["ncores"])))
    n, d = h.shape
    h_out = np.empty((n, d), np.float32)
    x_out = np.empty((n, DC), np.float32)
    cuts = cfg["cuts"]
    for k in range(cfg["ncores"]):
        lo_n, hi_n = cuts[k], cuts[k + 1]
        nreal = hi_n - lo_n
        h_out[lo_n:hi_n] = res.results[k]["hout"][:nreal]
        x_out[lo_n:hi_n] = res.results[k]["xout"][:nreal]
    return h_out, x_out


# revision 11
# speedup vs baseline: 1014.6194x; 1.0248x over previous
"""EGNN EquivariantBlock kernel for 8x TRN2 NeuronCores (Bass/Tile).

Strategy:
  - Sort edges by dst (host). Shard edges across 8 cores at node boundaries
    -> each core owns a contiguous dst-node range; no collectives needed.
  - Per core: dst-node blocks of 128 nodes. Edges of a block are split into
    lo/hi groups by src (so int16 dma_gather indices fit), padded to a
    uniform number of 128-edge subtiles per block (SPMD: same NEFF all cores).
  - Gather h||x rows (512B) by src via dma_gather. Edge MLPs run in
    feature-transposed orientation (features on partitions, edges on free):
      pre = Wa^T h_srcT + zb(dst, expanded via one-hot S^T) + Wra^T [r; a]
    Segment-sum via one-hot matmul (lhsT = S edge-row) into PSUM agg.
  - ACT engine runs ONLY silu (no table switches): sigmoid = silu(z)/z via
    DVE reciprocal; sqrt via DVE Newton-rsqrt bit trick.
  - Node phase per block: u = silu(cat(h, h_neigh) @ Wn1 + b), outputs
    h_out = h + u @ Wn2 + b, x_out = x + (agg_gx - x * agg_g).
"""
import numpy as np
import ml_dtypes

import concourse.bass as bass
import concourse.bacc as bacc
import concourse.mybir as mybir
import concourse.tile as tile
from concourse.bass_utils import run_bass_kernel_spmd
from concourse.library_config import mlp as _mlp_lib
from concourse.masks import make_identity

BF16 = mybir.dt.bfloat16
F32 = mybir.dt.float32
I32 = mybir.dt.int32
I16 = mybir.dt.int16
bf16 = ml_dtypes.bfloat16

# problem constants (hardcoded per contract)
N, E, D, DE, DC = 50000, 800000, 128, 16, 3
NCORES = 8
P = 128
_LAST = None


# ---------------------------------------------------------------- host prep
def _ceil(a, b):
    return -(-a // b)


def _wrap_idx(flat_idx):
    """dma_gather layout A: [n] int16 -> [128, n//16] (16-wrap, x8 replicate)."""
    base = flat_idx.reshape(-1, 16).T  # [16, n/16]
    return np.tile(base, (8, 1)).astype(np.int16)


def prep(h, x, a, src, dst, weights, ncores=NCORES):
    """Build per-core device arrays + config. weights: dict of W_*/b_* f32."""
    n, d = h.shape
    e = src.shape[0]
    de = a.shape[1]
    npad = _ceil(n, 256) * 256
    nlo = npad // 2
    src = src.astype(np.int64)
    dst = dst.astype(np.int64)

    # hx gather table: rows of 512B: h bf16[128] | x f32[3] (as 6 bf16 slots) | pad
    rowlen = 256  # bf16 elements
    hx = np.zeros((npad, rowlen), dtype=bf16)
    hx[:n, :d] = h.astype(bf16)
    hx[:n, d:d + 2 * DC] = x.astype(np.float32).view(np.uint16).reshape(n, 2 * DC).view(bf16)
    hx_lo, hx_hi = np.ascontiguousarray(hx[:nlo]), np.ascontiguousarray(hx[nlo:])

    perm = np.argsort(dst, kind="stable")
    dsts = dst[perm]
    srcs = src[perm]
    # shard cuts at node boundaries, ~equal edges
    counts = np.bincount(dst, minlength=n)
    cum = np.concatenate([[0], np.cumsum(counts)])  # cum[i] = #edges with dst < i
    cuts = [0]
    for k in range(1, ncores):
        tgt = e * k // ncores
        c = int(np.searchsorted(cum, tgt))
        c = min(max(c, cuts[-1] + 1), n - (ncores - k))
        cuts.append(c)
    cuts.append(n)
    nblk = max(_ceil(cuts[k + 1] - cuts[k], P) for k in range(ncores))

    # per (core, block) edge groups
    blocks = []  # (core, b) -> (lo_edges_idx, hi_edges_idx) positions into perm arrays
    max_lo = max_hi = 0
    for k in range(ncores):
        lo_n, hi_n = cuts[k], cuts[k + 1]
        for b in range(nblk):
            nb0 = lo_n + b * P
            nb1 = min(nb0 + P, hi_n)
            if nb0 >= hi_n:
                e0 = e1 = cum[hi_n]
            else:
                e0, e1 = cum[nb0], cum[nb1]
            seg = np.arange(e0, e1)
            is_lo = srcs[e0:e1] < nlo
            lo_i = seg[is_lo]
            hi_i = seg[~is_lo]
            blocks.append((lo_i, hi_i))
            max_lo = max(max_lo, len(lo_i))
            max_hi = max(max_hi, len(hi_i))
    lo_t = max(1, _ceil(max_lo, P))
    hi_t = max(1, _ceil(max_hi, P))
    nsub = lo_t + hi_t

    # chunk schedule: groups of up to 4 subtiles
    chunks = []
    s = 0
    while s < nsub:
        s1 = min(s + 4, nsub)
        chunks.append((s, s1))
        s = s1
    cfg = dict(n=n, d=d, e=e, de=de, npad=npad, nlo=nlo, nblk=nblk,
               lo_t=lo_t, hi_t=hi_t, nsub=nsub, chunks=chunks, cuts=cuts,
               ncores=ncores)

    in_maps = []
    af = a.astype(np.float32)
    for k in range(ncores):
        lo_n, hi_n = cuts[k], cuts[k + 1]
        nn = nblk * P
        gil = np.zeros((nblk, 128, lo_t * 8), np.int16)
        gih = np.zeros((nblk, 128, hi_t * 8), np.int16)
        dlrow = np.full((nblk, 1, nsub * P), -1.0, np.float32)
        dlcol = np.full((nblk, 128, nsub), -1.0, np.float32)
        aTt = np.zeros((nblk, 16, nsub * P), bf16)
        for b in range(nblk):
            lo_i, hi_i = blocks[k * nblk + b]
            base = lo_n + b * P
            il = np.zeros(lo_t * P, np.int64)
            il[:len(lo_i)] = srcs[lo_i]
            ih = np.full(hi_t * P, nlo, np.int64)
            ih[:len(hi_i)] = srcs[hi_i]
            gil[b] = _wrap_idx(il.astype(np.int16))
            gih[b] = _wrap_idx((ih - nlo).astype(np.int16))
            dl = np.full(nsub * P, -1.0, np.float32)
            dl[:len(lo_i)] = dsts[lo_i] - base
            dl[lo_t * P:lo_t * P + len(hi_i)] = dsts[hi_i] - base
            dlrow[b, 0] = dl
            dlcol[b] = dl.reshape(nsub, P).T
            av = np.zeros((nsub * P, de), np.float32)
            av[:len(lo_i)] = af[perm[lo_i]]
            av[lo_t * P:lo_t * P + len(hi_i)] = af[perm[hi_i]]
            aTt[b] = av.T.astype(bf16)
        hTt = np.zeros((128, nn), bf16)
        nodef = np.zeros((nn, 132), np.float32)
        nreal = hi_n - lo_n
        hTt[:, :nreal] = h[lo_n:hi_n].T.astype(bf16)
        nodef[:nreal, :d] = h[lo_n:hi_n]
        nodef[:nreal, d:d + DC] = x[lo_n:hi_n]
        w = weights
        im = {
            "hx_lo": hx_lo, "hx_hi": hx_hi, "hTt": hTt, "nodef": nodef,
            "gil": gil, "gih": gih, "dlrow": dlrow, "dlcol": dlcol, "aTt": aTt,
            "Wa_e1": w["W_e1"][:d].astype(bf16), "Wb_e1": w["W_e1"][d:2 * d].astype(bf16),
            "Wr_e1": w["W_e1"][2 * d:2 * d + 1].astype(bf16),
            "Waa_e1": w["W_e1"][2 * d + 1:].astype(bf16),
            "Wa_c1": w["W_c1"][:d].astype(bf16), "Wb_c1": w["W_c1"][d:2 * d].astype(bf16),
            "Wr_c1": w["W_c1"][2 * d:2 * d + 1].astype(bf16),
            "Waa_c1": w["W_c1"][2 * d + 1:].astype(bf16),
            "W_e2": w["W_e2"].astype(bf16), "W_c2": w["W_c2"].astype(bf16),
            "W_att": w["W_att"].astype(bf16), "W_c3": w["W_c3"].astype(bf16),
            "Wn1a": w["W_n1"][:d].astype(bf16), "Wn1b": w["W_n1"][d:].astype(bf16),
            "W_n2": w["W_n2"].astype(bf16),
            "b_e1": w["b_e1"].reshape(d, 1).astype(np.float32),
            "b_c1": w["b_c1"].reshape(d, 1).astype(np.float32),
            "b_e2": w["b_e2"].reshape(d, 1).astype(np.float32),
            "b_c2": w["b_c2"].reshape(d, 1).astype(np.float32),
            "b_n1": w["b_n1"].reshape(d, 1).astype(np.float32),
            "b_att": np.float32(w["b_att"]).reshape(1, 1) + np.float32(1e-30),
            "b_n2": w["b_n2"].reshape(1, d).astype(np.float32),
        }
        in_maps.append(im)
    return cfg, in_maps


# ---------------------------------------------------------------- device build
def build(cfg, repeat=1):
    d = cfg["d"]
    de = cfg["de"]
    nblk, lo_t, hi_t, nsub = cfg["nblk"], cfg["lo_t"], cfg["hi_t"], cfg["nsub"]
    chunks = cfg["chunks"]
    nlo = cfg["nlo"]
    npad = cfg["npad"]
    nn = nblk * P
    AluOp = mybir.AluOpType
    SILU = mybir.ActivationFunctionType.Silu

    nc = bacc.Bacc("TRN2", target_bir_lowering=False, debug=False)
    dt_in = {
        "hx_lo": ([nlo, 256], BF16), "hx_hi": ([npad - nlo, 256], BF16),
        "hTt": ([128, nn], BF16), "nodef": ([nn, 132], F32),
        "gil": ([nblk, 128, lo_t * 8], I16), "gih": ([nblk, 128, hi_t * 8], I16),
        "dlrow": ([nblk, 1, nsub * P], F32), "dlcol": ([nblk, 128, nsub], F32),
        "aTt": ([nblk, 16, nsub * P], BF16),
        "Wa_e1": ([d, d], BF16), "Wb_e1": ([d, d], BF16), "Wr_e1": ([1, d], BF16),
        "Waa_e1": ([de, d], BF16),
        "Wa_c1": ([d, d], BF16), "Wb_c1": ([d, d], BF16), "Wr_c1": ([1, d], BF16),
        "Waa_c1": ([de, d], BF16),
        "W_e2": ([d, d], BF16), "W_c2": ([d, d], BF16),
        "W_att": ([d, 1], BF16), "W_c3": ([d, 1], BF16),
        "Wn1a": ([d, d], BF16), "Wn1b": ([d, d], BF16), "W_n2": ([d, d], BF16),
        "b_e1": ([d, 1], F32), "b_c1": ([d, 1], F32), "b_e2": ([d, 1], F32),
        "b_c2": ([d, 1], F32), "b_n1": ([d, 1], F32), "b_att": ([1, 1], F32),
        "b_n2": ([1, d], F32),
    }
    H = {k: nc.dram_tensor(k, shp, t, kind="ExternalInput") for k, (shp, t) in dt_in.items()}
    hout = nc.dram_tensor("hout", [nn, d], F32, kind="ExternalOutput")
    xout = nc.dram_tensor("xout", [nn, DC], F32, kind="ExternalOutput")

    with tile.TileContext(nc) as tc:
        with tc.tile_critical():
            nc.gpsimd.load_library(_mlp_lib)
        with tc.tile_pool(name="wp", bufs=1) as wp, \
             tc.tile_pool(name="sp", bufs=2) as sp, \
             tc.tile_pool(name="pp", bufs=2, space="PSUM") as pp:
            # ---- constants
            W = {}
            for k in ["Wa_e1", "Wb_e1", "Wr_e1", "Waa_e1", "Wa_c1", "Wb_c1",
                      "Wr_c1", "Waa_c1", "W_e2", "W_c2", "W_att", "W_c3",
                      "Wn1a", "Wn1b", "W_n2"]:
                t = wp.tile(dt_in[k][0], BF16, tag=k)
                nc.sync.dma_start(out=t[:], in_=H[k][:])
                W[k] = t
            B = {}
            for k in ["b_e1", "b_c1", "b_e2", "b_c2", "b_n1", "b_att"]:
                t = wp.tile(dt_in[k][0], F32, tag=k)
                nc.sync.dma_start(out=t[:], in_=H[k][:])
                B[k] = t
            b_att_col = wp.tile([128, 1], F32, tag="b_att_col")
            nc.gpsimd.partition_broadcast(b_att_col[:], B["b_att"][:])
            b_n2row = wp.tile([1, d], F32, tag="b_n2row")
            nc.sync.dma_start(out=b_n2row[:], in_=H["b_n2"][:])
            b_n2bc = wp.tile([128, d], F32, tag="b_n2bc")
            nc.gpsimd.partition_broadcast(b_n2bc[:], b_n2row[:])
            idn_b = wp.tile([128, 128], BF16, tag="idn_b")
            make_identity(nc, idn_b[:])
            idn_f = wp.tile([128, 128], F32, tag="idn_f")
            make_identity(nc, idn_f[:])
            idn33_f = wp.tile([33, 33], F32, tag="idn33_f")
            make_identity(nc, idn33_f[:])
            iota_c = wp.tile([128, 1], I32, tag="iota_c")
            nc.gpsimd.iota(iota_c[:], pattern=[[0, 1]], base=0, channel_multiplier=1)
            iota_cf = wp.tile([128, 1], F32, tag="iota_cf")
            nc.vector.tensor_copy(iota_cf[:], iota_c[:])
            iota_r = wp.tile([128, 128], I32, tag="iota_r")
            nc.gpsimd.iota(iota_r[:], pattern=[[1, 128]], base=0, channel_multiplier=0)
            iota_rf = wp.tile([128, 128], F32, tag="iota_rf")
            nc.vector.tensor_copy(iota_rf[:], iota_r[:])

            import contextlib
            loop_ctx = tc.For_i(0, repeat, 1) if repeat > 1 else contextlib.nullcontext()
            with loop_ctx:
              for b in range(nblk):
                # ---------------- stage A: node block prep
                hTblk = sp.tile([128, 128], BF16, tag="hTblk")
                nc.sync.dma_start(out=hTblk[:], in_=H["hTt"][:, b * P:(b + 1) * P])
                nodeblk = sp.tile([128, 132], F32, tag="nodeblk")
                nc.sync.dma_start(out=nodeblk[:], in_=H["nodef"][b * P:(b + 1) * P, :])
                xblk_b = sp.tile([128, DC], BF16, tag="xblk_b")
                nc.vector.tensor_copy(xblk_b[:], nodeblk[:, d:d + DC])
                zb_ps = pp.tile([128, 256], F32, tag="scr2", bufs=1)
                nc.tensor.matmul(out=zb_ps[:, 0:128], lhsT=hTblk[:], rhs=W["Wb_e1"][:],
                                 start=True, stop=True)
                nc.tensor.matmul(out=zb_ps[:, 128:256], lhsT=hTblk[:], rhs=W["Wb_c1"][:],
                                 start=True, stop=True)
                zbe = sp.tile([128, 128], BF16, tag="zbe")
                nc.vector.tensor_copy(zbe[:], zb_ps[:, 0:128])
                zbc = sp.tile([128, 128], BF16, tag="zbc")
                nc.vector.tensor_copy(zbc[:], zb_ps[:, 128:256])

                # ---------------- stage B: gather + geometry
                gi_l = sp.tile([128, lo_t * 8], I16, tag="gi_l")
                nc.sync.dma_start(out=gi_l[:], in_=H["gil"][b])
                gi_h = sp.tile([128, hi_t * 8], I16, tag="gi_h")
                nc.sync.dma_start(out=gi_h[:], in_=H["gih"][b])
                gat = sp.tile([128, nsub, 256], BF16, tag="gat", bufs=3)
                nc.gpsimd.dma_gather(
                    out_ap=gat[:, 0:lo_t, :], in_ap=H["hx_lo"][:], idxs_ap=gi_l[:],
                    num_idxs=lo_t * P, num_idxs_reg=lo_t * P, elem_size=256,
                    single_packet=(lo_t * P <= 1024))
                nc.gpsimd.dma_gather(
                    out_ap=gat[:, lo_t:nsub, :], in_ap=H["hx_hi"][:], idxs_ap=gi_h[:],
                    num_idxs=hi_t * P, num_idxs_reg=hi_t * P, elem_size=256,
                    single_packet=(hi_t * P <= 1024))
                dlr = sp.tile([1, nsub * P], F32, tag="dlr")
                nc.sync.dma_start(out=dlr[:], in_=H["dlrow"][b])
                dlc = sp.tile([128, nsub], F32, tag="dlc")
                nc.sync.dma_start(out=dlc[:], in_=H["dlcol"][b])
                aT = sp.tile([16, nsub * P], BF16, tag="aT")
                nc.sync.dma_start(out=aT[:], in_=H["aTt"][b])
                # S^T [nodes(P), edges] and S_sub [edges(P), nodes]
                dlb = sp.tile([128, nsub * P], F32, tag="dlb")
                nc.gpsimd.partition_broadcast(dlb[:], dlr[:])
                st_all = sp.tile([128, nsub * P], BF16, tag="st_all")
                nc.vector.tensor_scalar(out=st_all[:], in0=dlb[:],
                                        scalar1=iota_cf[:, 0:1], scalar2=None,
                                        op0=AluOp.is_equal)
                ssub = sp.tile([128, nsub, 128], BF16, tag="ssub")
                nc.vector.tensor_tensor(
                    out=ssub[:],
                    in0=iota_rf[:].rearrange("p (o f) -> p o f", o=1).to_broadcast([128, nsub, 128]),
                    in1=dlc[:].rearrange("p (t o) -> p t o", o=1).to_broadcast([128, nsub, 128]),
                    op=AluOp.is_equal)
                # x_dst per subtile -> xdp psum [128, 3t]
                xdp = pp.tile([128, 3 * nsub], F32, tag="scr", bufs=1)
                for t in range(nsub):
                    nc.tensor.matmul(out=xdp[:, 3 * t:3 * t + 3],
                                     lhsT=st_all[:, t * P:(t + 1) * P], rhs=xblk_b[:],
                                     start=True, stop=True)
                rcols = sp.tile([128, nsub], F32, tag="rcols")
                xdsc = sp.tile([128, nsub, DC], F32, tag="xdsc")
                nc.vector.tensor_tensor(out=xdsc[:], in0=gat[:, :, d:d + 2 * DC].bitcast(F32),
                                        in1=xdp[:].rearrange("p (t c) -> p t c", c=3),
                                        op=AluOp.subtract)
                nc.vector.tensor_tensor(out=xdsc[:], in0=xdsc[:], in1=xdsc[:],
                                        op=AluOp.mult)
                nc.vector.reduce_sum(rcols[:].rearrange("p (t o) -> p t o", o=1), xdsc[:],
                                     axis=mybir.AxisListType.X)
                # radial = sqrt(rcols) via Newton rsqrt; invr = 1/(1+radial)
                rc = sp.tile([128, nsub], F32, tag="rc")
                nc.vector.tensor_scalar(out=rc[:], in0=rcols[:], scalar1=1e-20,
                                        scalar2=None, op0=AluOp.max)
                ybit = sp.tile([128, nsub], I32, tag="ybit")
                nc.vector.tensor_scalar(out=ybit[:], in0=rc[:].bitcast(I32), scalar1=1,
                                        scalar2=None, op0=AluOp.logical_shift_right)
                nc.vector.tensor_scalar(out=ybit[:], in0=ybit[:], scalar1=-1,
                                        scalar2=0x5f3759df, op0=AluOp.mult, op1=AluOp.add)
                ny = sp.tile([128, nsub], F32, tag="ny")
                nc.vector.tensor_copy(out=ny[:], in_=ybit[:].bitcast(F32))
                nxh = sp.tile([128, nsub], F32, tag="nxh")
                nc.vector.tensor_scalar(out=nxh[:], in0=rc[:], scalar1=0.5, scalar2=None,
                                        op0=AluOp.mult)
                nt = sp.tile([128, nsub], F32, tag="nt")
                for _ in range(3):
                    nc.vector.tensor_tensor(out=nt[:], in0=ny[:], in1=ny[:], op=AluOp.mult)
                    nc.vector.tensor_tensor(out=nt[:], in0=nt[:], in1=nxh[:], op=AluOp.mult)
                    nc.vector.tensor_scalar(out=nt[:], in0=nt[:], scalar1=-1.0, scalar2=1.5,
                                            op0=AluOp.mult, op1=AluOp.add)
                    nc.vector.tensor_tensor(out=ny[:], in0=ny[:], in1=nt[:], op=AluOp.mult)
                rad = sp.tile([128, nsub], F32, tag="rad")
                nc.vector.tensor_tensor(out=rad[:], in0=rcols[:], in1=ny[:], op=AluOp.mult)
                invr = sp.tile([128, nsub], F32, tag="invr")
                nc.vector.tensor_scalar(out=invr[:], in0=rad[:], scalar1=1.0, scalar2=None,
                                        op0=AluOp.add)
                nc.vector.reciprocal(invr[:], invr[:])
                radT = sp.tile([1, nsub * P], BF16, tag="radT")

                # agg psum for the whole block
                agg = pp.tile([128, 132], F32, tag="agg", bufs=1)
                first_seg = True

                # ---------------- stage C: MLP chunks
                for (s0, s1) in chunks:
                    ns = s1 - s0
                    ef = ns * P
                    hsT_ps = pp.tile([128, 4 * P], BF16, tag="trh", bufs=1)
                    for t in range(ns):
                        nc.tensor.transpose(out=hsT_ps[:, t * P:(t + 1) * P],
                                            in_=gat[:, s0 + t, 0:d], identity=idn_b[:])
                    hsrcT = sp.tile([128, 4 * P], BF16, tag="hsrcT", bufs=3)
                    nc.vector.tensor_copy(hsrcT[:, 0:ef], hsT_ps[:, 0:ef])
                    radT_ps = pp.tile([1, 4 * P], F32, tag="scr3", bufs=1)
                    for t in range(ns):
                        nc.tensor.transpose(out=radT_ps[0:1, t * P:(t + 1) * P],
                                            in_=rad[:, s0 + t:s0 + t + 1], identity=idn_f[:])
                    nc.vector.tensor_copy(radT[0:1, s0 * P:s1 * P], radT_ps[0:1, 0:ef])
                    for (nm, wa, zb, wr, waa, bb) in [
                            ("e1", "Wa_e1", zbe, "Wr_e1", "Waa_e1", "b_e1"),
                            ("c1", "Wa_c1", zbc, "Wr_c1", "Waa_c1", "b_c1")]:
                        pre = pp.tile([128, 4 * P], F32, tag="mlp", bufs=2)
                        nc.tensor.matmul(out=pre[:, 0:ef], lhsT=W[wa][:], rhs=hsrcT[:, 0:ef],
                                         start=True, stop=False)
                        nc.tensor.matmul(out=pre[:, 0:ef], lhsT=zb[:],
                                         rhs=st_all[:, s0 * P:s1 * P], start=False, stop=False)
                        nc.tensor.matmul(out=pre[:, 0:ef], lhsT=W[waa][:],
                                         rhs=aT[:, s0 * P:s1 * P], start=False, stop=False)
                        nc.tensor.matmul(out=pre[:, 0:ef], lhsT=W[wr][:],
                                         rhs=radT[0:1, s0 * P:s1 * P], start=False,
                                         stop=True)
                        mt = sp.tile([128, 4 * P], BF16, tag=f"m_{nm}", bufs=3)
                        nc.scalar.activation(mt[:, 0:ef], pre[:, 0:ef], SILU, bias=B[bb][:, 0:1])
                        if nm == "e1":
                            m1T = mt
                        else:
                            c1T = mt
                    z2 = pp.tile([128, 4 * P], F32, tag="mlp", bufs=2)
                    nc.tensor.matmul(out=z2[:, 0:ef], lhsT=W["W_e2"][:], rhs=m1T[:, 0:ef],
                                     start=True, stop=True)
                    msghT = sp.tile([128, 4 * P], BF16, tag="msghT", bufs=3)
                    nc.scalar.activation(msghT[:, 0:ef], z2[:, 0:ef], SILU, bias=B["b_e2"][:, 0:1])
                    z2c = pp.tile([128, 4 * P], F32, tag="mlp", bufs=2)
                    nc.tensor.matmul(out=z2c[:, 0:ef], lhsT=W["W_c2"][:], rhs=c1T[:, 0:ef],
                                     start=True, stop=True)
                    c2T = sp.tile([128, 4 * P], BF16, tag="c2T", bufs=3)
                    nc.scalar.activation(c2T[:, 0:ef], z2c[:, 0:ef], SILU, bias=B["b_c2"][:, 0:1])
                    # att raw + cw rows
                    ac_ps = pp.tile([33, 4 * P], F32, tag="scr3", bufs=1)
                    nc.tensor.matmul(out=ac_ps[0:1, 0:ef], lhsT=W["W_att"][:],
                                     rhs=msghT[:, 0:ef], start=True, stop=True)
                    nc.tensor.matmul(out=ac_ps[32:33, 0:ef], lhsT=W["W_c3"][:],
                                     rhs=c2T[:, 0:ef], start=True, stop=True)
                    pack2 = sp.tile([33, 4 * P], F32, tag="pack2")
                    nc.vector.tensor_copy(pack2[0:33, 0:ef], ac_ps[0:33, 0:ef])
                    p2T_ps = pp.tile([128, 4 * 33], F32, tag="scr2", bufs=1)
                    for t in range(ns):
                        nc.tensor.transpose(out=p2T_ps[:, 33 * t:33 * t + 33],
                                            in_=pack2[:, t * P:(t + 1) * P], identity=idn33_f[:])
                    # sigmoid(att+b) = silu(w)/w
                    wcol = sp.tile([128, 4], F32, tag="wcol")
                    nc.vector.tensor_scalar(out=wcol[:, 0:ns],
                                            in0=p2T_ps[:, 0:33 * ns:33],
                                            scalar1=b_att_col[:, 0:1],
                                            scalar2=None, op0=AluOp.add)
                    silw = sp.tile([128, 4], F32, tag="silw")
                    nc.scalar.activation(silw[:, 0:ns], wcol[:, 0:ns], SILU)
                    rw = sp.tile([128, 4], F32, tag="rw")
                    nc.vector.reciprocal(rw[:, 0:ns], wcol[:, 0:ns])
                    att = sp.tile([128, 4], F32, tag="att")
                    nc.vector.tensor_tensor(out=att[:, 0:ns], in0=silw[:, 0:ns],
                                            in1=rw[:, 0:ns], op=AluOp.mult)
                    gcol = sp.tile([128, 4], F32, tag="gcol")
                    nc.vector.tensor_tensor(out=gcol[:, 0:ns], in0=p2T_ps[:, 32:33 * ns:33],
                                            in1=invr[:, s0:s1], op=AluOp.mult)
                    # per subtile: transpose msg, gate, pack, segsum
                    mge_ps = pp.tile([128, 4 * P], BF16, tag="trm", bufs=1)
                    for t in range(ns):
                        nc.tensor.transpose(out=mge_ps[:, t * P:(t + 1) * P],
                                            in_=msghT[:, t * P:(t + 1) * P], identity=idn_b[:])
                    gp = sp.tile([128, 4, 132], BF16, tag="gp", bufs=2)
                    nc.vector.tensor_tensor(
                        out=gp[:, 0:ns, 0:d],
                        in0=mge_ps[:, 0:ef].rearrange("p (t f) -> p t f", f=P),
                        in1=att[:, 0:ns].rearrange("p (t o) -> p t o", o=1).to_broadcast([128, ns, d]),
                        op=AluOp.mult)
                    nc.vector.tensor_tensor(
                        out=gp[:, 0:ns, d:d + DC],
                        in0=gat[:, s0:s1, d:d + 2 * DC].bitcast(F32),
                        in1=gcol[:, 0:ns].rearrange("p (t o) -> p t o", o=1).to_broadcast([128, ns, DC]),
                        op=AluOp.mult)
                    nc.vector.tensor_copy(gp[:, 0:ns, d + DC:d + DC + 1],
                                          gcol[:, 0:ns].rearrange("p (t o) -> p t o", o=1))
                    for t in range(ns):
                        nc.tensor.matmul(out=agg[:], lhsT=ssub[:, s0 + t, :], rhs=gp[:, t, :],
                                         start=first_seg, stop=(s0 + t == nsub - 1))
                        first_seg = False

                # ---------------- stage D: node update
                aggs = sp.tile([128, 132], F32, tag="aggs")
                nc.vector.tensor_copy(aggs[:], agg[:])
                hnb = sp.tile([128, 128], BF16, tag="hnb")
                nc.vector.tensor_copy(hnb[:], aggs[:, 0:d])
                hnT_ps = pp.tile([128, 128], BF16, tag="scr3", bufs=1)
                nc.tensor.transpose(out=hnT_ps[:], in_=hnb[:], identity=idn_b[:])
                hnT = sp.tile([128, 128], BF16, tag="hnT")
                nc.vector.tensor_copy(hnT[:], hnT_ps[:])
                upre = pp.tile([128, 128], F32, tag="scr", bufs=1)
                nc.tensor.matmul(out=upre[:], lhsT=W["Wn1a"][:], rhs=hTblk[:],
                                 start=True, stop=False)
                nc.tensor.matmul(out=upre[:], lhsT=W["Wn1b"][:], rhs=hnT[:],
                                 start=False, stop=True)
                uT = sp.tile([128, 128], BF16, tag="uT")
                nc.scalar.activation(uT[:], upre[:], SILU, bias=B["b_n1"][:, 0:1])
                hdel = pp.tile([128, 128], F32, tag="scr2", bufs=1)
                nc.tensor.matmul(out=hdel[:], lhsT=uT[:], rhs=W["W_n2"][:],
                                 start=True, stop=True)
                hout_s = sp.tile([128, 128], F32, tag="hout_s")
                nc.vector.tensor_tensor(out=hout_s[:], in0=hdel[:], in1=nodeblk[:, 0:d],
                                        op=AluOp.add)
                nc.vector.tensor_tensor(out=hout_s[:], in0=hout_s[:], in1=b_n2bc[:],
                                        op=AluOp.add)
                nc.sync.dma_start(out=hout[b * P:(b + 1) * P, :], in_=hout_s[:])
                xout_s = sp.tile([128, DC], F32, tag="xout_s")
                nc.vector.tensor_scalar(out=xout_s[:], in0=nodeblk[:, d:d + DC],
                                        scalar1=aggs[:, d + DC:d + DC + 1], scalar2=None,
                                        op0=AluOp.mult)
                nc.vector.tensor_tensor(out=xout_s[:], in0=aggs[:, d:d + DC], in1=xout_s[:],
                                        op=AluOp.subtract)
                nc.vector.tensor_tensor(out=xout_s[:], in0=xout_s[:], in1=nodeblk[:, d:d + DC],
                                        op=AluOp.add)
                nc.sync.dma_start(out=xout[b * P:(b + 1) * P, :], in_=xout_s[:])

    nc.compile()
    return nc


# ---------------------------------------------------------------- entry point
def kernel(h, x, a, src, dst,
           W_e1, b_e1, W_e2, b_e2, W_att, b_att,
           W_n1, b_n1, W_n2, b_n2,
           W_c1, b_c1, W_c2, b_c2, W_c3):
    h = np.asarray(h, np.float32)
    x = np.asarray(x, np.float32)
    a = np.asarray(a, np.float32)
    weights = dict(W_e1=np.asarray(W_e1, np.float32), b_e1=np.asarray(b_e1, np.float32),
                   W_e2=np.asarray(W_e2, np.float32), b_e2=np.asarray(b_e2, np.float32),
                   W_att=np.asarray(W_att, np.float32), b_att=np.asarray(b_att, np.float32),
                   W_n1=np.asarray(W_n1, np.float32), b_n1=np.asarray(b_n1, np.float32),
                   W_n2=np.asarray(W_n2, np.float32), b_n2=np.asarray(b_n2, np.float32),
                   W_c1=np.asarray(W_c1, np.float32), b_c1=np.asarray(b_c1, np.float32),
                   W_c2=np.asarray(W_c2, np.float32), b_c2=np.asarray(b_c2, np.float32),
                   W_c3=np.asarray(W_c3, np.float32))
    cfg, in_maps = prep(h, x, a, np.asarray(src), np.asarray(dst), weights)
    nc = build(cfg)
    global _LAST
    _LAST = (cfg, in_maps, nc)
    res = run_bass_kernel_spmd(nc, in_maps, core_ids=list(range(cfg["ncores"])))
    n, d = h.shape
    h_out = np.empty((n, d), np.float32)
    x_out = np.empty((n, DC), np.float32)
    cuts = cfg["cuts"]
    for k in range(cfg["ncores"]):
        lo_n, hi_n = cuts[k], cuts[k + 1]
        nreal = hi_n - lo_n
        h_out[lo_n:hi_n] = res.results[k]["hout"][:nreal]
        x_out[lo_n:hi_n] = res.results[k]["xout"][:nreal]
    return h_out, x_out
